# revision 1
# baseline (speedup 1.0000x reference)
"""Trainium2 Bass kernel for nn_CausalRankKAttention.

Blend of banded-softmax attention and cumsum linear attention, per (n,h) pair.
16 pairs sharded over 8 NeuronCores (2 pairs/core), no cross-core comm.

Design (v6):
  - feature map phi(x)=tanh(x)+1 on HOST; only the exp table ever loads on ACT.
  - all matmuls bf16 (PE streams ~1 col/cycle at 1.2GHz regardless of dtype;
    fp8 DoubleRow measured no faster).
  - per block lb (ascending, = linear chunk lb):
      tail: forward scores q-block lb vs s-blocks [0, lb) -> wide psum
        [128, lb*128], one ACT exp with fused accum_out -> denominator tail.
      band: transposed tile st[k=lb, q in lb..lb+1] -> exp -> mask.
      linear: transposed scores -> causal mask -> kn state delta.
  - key trick: with a binary key mask, vsm == vlin == [v, klm]; the band "mv"
    matmul and the linear "atv" matmul share their stationary tensor, so one
    384-col matmul computes both (rhs = [st_m | at] written side by side by
    DVE), with the inter matmul accumulating into the last 128 cols of the
    same psum group. 5 small matmuls per iteration instead of 7.
  - dependent matmuls trail one iteration; band+linear share one [128, 1024]
    psum tile per iteration, drained by a single [65, 384] CAST.
  - outputs are RAW numerators/denominators; normalize + blend on host.
"""

import numpy as np
import ml_dtypes

import concourse.bass as bass
import concourse.bacc as bacc
import concourse.mybir as mybir
import concourse.tile as tile
from concourse import bass_utils

F32 = mybir.dt.float32
BF16 = mybir.dt.bfloat16
AF = mybir.ActivationFunctionType
OP = mybir.AluOpType

N, L, H, E = 2, 2048, 8, 64
NB = L // 128            # 16 blocks/chunks of 128
TEMP = float(1.0 / np.sqrt(E))
EPS = 1e-6
PAIRS_PER_CORE = 2
NCORES = 8

_cached = {}


def build_nc():
    nc = bacc.Bacc("TRN2", target_bir_lowering=False, debug=False,
                   num_devices=NCORES)
    P = PAIRS_PER_CORE
    # ---- dram tensors (per core) ----
    # qkt[p, :, 0] = kt (k^T + gate ext row), [p, :, 1] = qt (q^T + ones row)
    qkt = nc.dram_tensor("qkt", [P, 65, 2, L], BF16, kind="ExternalInput")
    sg = nc.dram_tensor("sg", [P, 64, 2, L], BF16, kind="ExternalInput")
    # vvkn: [vv (NB*65) | sgkn (NB*64)]
    vvkn = nc.dram_tensor("vvkn", [P, 128, NB * 65 + NB * 64], BF16,
                          kind="ExternalInput")
    m01d = nc.dram_tensor("m01d", [128, 256], BF16, kind="ExternalInput")
    svlv = nc.dram_tensor("svlv", [P, 65, NB, 384], BF16, kind="ExternalOutput")
    # two accumulator columns per block (tail split in halves); host sums
    tails = nc.dram_tensor("tails", [P, 128, 2 * NB], F32, kind="ExternalOutput")

    with tile.TileContext(nc) as tc:
        with (
            tc.tile_pool(name="const", bufs=1) as constp,
            tc.tile_pool(name="io", bufs=2) as iop,
            tc.tile_pool(name="acc", bufs=2) as accp,
            tc.tile_pool(name="work", bufs=2) as workp,
            tc.tile_pool(name="sp", bufs=3) as sp,
            tc.tile_pool(name="tailp", bufs=1, space="PSUM") as tailp,
            tc.tile_pool(name="scp", bufs=2, space="PSUM") as scp,
            tc.tile_pool(name="otp", bufs=2, space="PSUM") as otp,
        ):
            m01_sb = constp.tile([128, 256], BF16, tag="m01")
            nc.sync.dma_start(m01_sb[:], m01d[:])

            # ---- input DMAs, pair-interleaved so both pairs start fast ----
            qkt_sbs, sg_sbs, vvkn_sbs, accs, tacc = [], [], [], [], []
            for p in range(P):
                qkt_sb = iop.tile([65, 2, L], BF16, tag=f"qkt{p}")
                sg_sb = iop.tile([64, 2, L], BF16, tag=f"sg{p}")
                vvkn_sb = iop.tile([128, NB * 65 + NB * 64], BF16,
                                   tag=f"vvkn{p}")
                qkt_sbs.append(qkt_sb)
                sg_sbs.append(sg_sb)
                vvkn_sbs.append(vvkn_sb)
                a = accp.tile([65, NB, 384], BF16, tag=f"acc{p}")
                t = accp.tile([128, 2 * NB], F32, tag=f"tails{p}")
                nc.gpsimd.memset(t[:], 0.0)
                accs.append(a); tacc.append(t)
            for p in range(P):
                nc.sync.dma_start(qkt_sbs[p][:, :, 0:1024], qkt[p, :, :, 0:1024])
                nc.sync.dma_start(qkt_sbs[p][:, :, 1024:2048],
                                  qkt[p, :, :, 1024:2048])
            for p in range(P):
                nc.sync.dma_start(sg_sbs[p][:, :, 0:1024], sg[p, :, :, 0:1024])
            for p in range(P):
                nc.sync.dma_start(vvkn_sbs[p][:], vvkn[p])
            for p in range(P):
                nc.sync.dma_start(sg_sbs[p][:, :, 1024:2048],
                                  sg[p, :, :, 1024:2048])

            def vv_ap(p, i):
                return vvkn_sbs[p][:, i * 65:(i + 1) * 65]

            def sgkn_ap(p, c):
                return vvkn_sbs[p][:, NB * 65 + c * 64:NB * 65 + (c + 1) * 64]

            # per-pair persistent [128, 1024] tail psum tiles (2 banks each)
            tp0 = tailp.tile([128, 1024], F32, tag="tp0")
            tp1 = tailp.tile([128, 1024], F32, tag="tp1")
            tps = [tp0, tp1]

            def tail_pass(p, i, lo, hi, acccol):
                """scores q-block i vs s in [lo, hi) -> exp+accum into col."""
                tp_ = tps[p]
                q0 = i * 128
                qt_sb = qkt_sbs[p][:, 1, :]
                kt_sb = qkt_sbs[p][:, 0, :]
                for off in range(lo, hi, 512):
                    n_ = min(512, hi - off)
                    nc.tensor.matmul(tp_[:, off - lo:off - lo + n_],
                                     qt_sb[:, q0:q0 + 128],
                                     kt_sb[:, off:off + n_],
                                     start=True, stop=True)
                scr = workp.tile([128, 1024], BF16, tag=f"scrap{p}")
                nc.scalar.activation(scr[:, 0:hi - lo], tp_[:, 0:hi - lo],
                                     AF.Exp, scale=TEMP,
                                     accum_out=tacc[p][:, acccol:acccol + 1])

            def tail_block(p, i):
                w = i * 128
                tail_pass(p, i, 0, min(w, 1024), i)
                if w > 1024:
                    tail_pass(p, i, 1024, w, NB + i)

            # hoisted: block-15 tails for both pairs, pass-interleaved
            for p in range(P):
                tail_pass(p, NB - 1, 0, 1024, NB - 1)
            for p in range(P):
                tail_pass(p, NB - 1, 1024, (NB - 1) * 128, 2 * NB - 1)

            s_cur = [None, None]
            prev = [None, None]
            for n in range(NB):
                for p in range(P):
                    qw = 256 if n < NB - 1 else 128
                    c0, c1 = n * 128, (n + 1) * 128
                    kt_sb = qkt_sbs[p][:, 0, :]
                    qt_sb = qkt_sbs[p][:, 1, :]
                    sgk_sb = sg_sbs[p][:, 0, :]
                    sgq_sb = sg_sbs[p][:, 1, :]

                    sc = scp.tile([128, 512], F32, tag="sc")
                    nc.tensor.matmul(sc[:, 0:qw], kt_sb[:, c0:c1],
                                     qt_sb[:, c0:c0 + qw],
                                     start=True, stop=True)
                    nc.tensor.matmul(sc[:, 256:384], sgk_sb[:, c0:c1],
                                     sgq_sb[:, c0:c1], start=True, stop=True)
                    if prev[p] is not None:
                        pot = prev[p]["ot"]
                        pn = n - 1
                        nc.tensor.matmul(pot[:], vv_ap(p, pn),
                                         prev[p]["stat"][:], start=True,
                                         stop=(pn == 0), skip_group_check=True)
                        if pn > 0:
                            nc.tensor.matmul(pot[:, 256:384],
                                             prev[p]["s_before"][:],
                                             sgq_sb[:, pn * 128:pn * 128 + 128],
                                             start=False, stop=True,
                                             skip_group_check=True)
                        nc.vector.tensor_copy(accs[p][:, pn, :], pot[:])
                    nc.tensor.matmul(sc[0:64, 384:449], sgkn_ap(p, n),
                                     vv_ap(p, n), start=True, stop=True)

                    st_e = workp.tile([128, 256], BF16, tag="st_e")
                    nc.scalar.activation(st_e[:, 0:qw], sc[:, 0:qw], AF.Exp,
                                         scale=TEMP)
                    if 1 <= n < NB - 1:
                        tail_block(p, n)

                    stat = workp.tile([128, 384], BF16, tag="stat")
                    nc.vector.tensor_tensor(stat[:, 256:384], sc[:, 256:384],
                                            m01_sb[:, 0:128], OP.mult)
                    nc.vector.tensor_tensor(stat[:, 0:qw], st_e[:, 0:qw],
                                            m01_sb[:, 0:qw], OP.mult)
                    if qw < 256:
                        nc.vector.memset(stat[:, 128:256], 0.0)
                    s_before = s_cur[p]
                    s_nxt = sp.tile([64, 65], BF16, tag=f"s{p}")
                    if n == 0:
                        nc.vector.tensor_copy(s_nxt[:], sc[0:64, 384:449])
                    else:
                        nc.vector.scalar_tensor_tensor(s_nxt[:], s_cur[p][:], 1.0,
                                                       sc[0:64, 384:449],
                                                       OP.mult, OP.add)
                    s_cur[p] = s_nxt

                    ot = otp.tile([65, 384], F32, tag="ot")
                    prev[p] = {"ot": ot, "stat": stat, "s_before": s_before}
                    if n in (5, 9, 13):
                        nc.sync.dma_start(svlv[p, :, n - 5:n - 1, :],
                                          accs[p][:, n - 5:n - 1, :])

            # ---- epilogue: finish last block for both pairs ----
            for p in range(P):
                sgq_sb = sg_sbs[p][:, 1, :]
                pot = prev[p]["ot"]
                pn = NB - 1
                nc.tensor.matmul(pot[:], vv_ap(p, pn), prev[p]["stat"][:],
                                 start=True, stop=False, skip_group_check=True)
                nc.tensor.matmul(pot[:, 256:384], prev[p]["s_before"][:],
                                 sgq_sb[:, pn * 128:pn * 128 + 128],
                                 start=False, stop=True, skip_group_check=True)
                nc.vector.tensor_copy(accs[p][:, pn, :], pot[:])
                nc.scalar.dma_start(svlv[p, :, 12:NB, :], accs[p][:, 12:NB, :])
                nc.scalar.dma_start(tails[p], tacc[p][:])

    nc.compile()
    return nc


def host_prep(queries, keys, values, key_lengths_mask, blend):
    """Build per-core in_maps from full inputs."""
    q = np.ascontiguousarray(np.transpose(queries, (0, 2, 1, 3)))  # [N,H,L,E]
    k = np.ascontiguousarray(np.transpose(keys, (0, 2, 1, 3)))
    v = np.ascontiguousarray(np.transpose(values, (0, 2, 1, 3)))
    q = q.reshape(N * H, L, E).astype(np.float32)
    k = k.reshape(N * H, L, E).astype(np.float32)
    v = v.reshape(N * H, L, E).astype(np.float32)
    klm = np.asarray(key_lengths_mask, np.float32)  # [N, L]

    ii = np.arange(128)[:, None]
    cc = np.arange(256)[None, :]
    m01 = ((cc - ii >= 0) & (cc - ii <= 128)).astype(np.float32)

    in_maps = []
    for core in range(NCORES):
        qkts, sgs, vvkns = [], [], []
        for p in range(PAIRS_PER_CORE):
            g = core * PAIRS_PER_CORE + p
            n = g // H
            qg, kg, vg = q[g], k[g], v[g]          # [L, E]
            kl = klm[n]                             # [L]
            i01 = (kl > 0).astype(np.float32)

            qkt_p = np.empty((65, 2, L), np.float32)
            qkt_p[0:64, 0] = kg.T
            qkt_p[64, 0] = -1e9 * (1.0 - i01)
            qkt_p[0:64, 1] = qg.T
            qkt_p[64, 1] = 1.0

            phiq = np.tanh(qg) + 1.0
            phik = np.tanh(kg) + 1.0
            sg_p = np.empty((64, 2, L), np.float32)
            sg_p[:, 0] = phik.T
            sg_p[:, 1] = phiq.T

            vv_full = np.empty((L, 65), np.float32)
            vv_full[:, 0:64] = vg * kl[:, None]
            vv_full[:, 64] = kl
            vv_p = vv_full.reshape(NB, 128, 65).transpose(1, 0, 2)
            sgkn_p = phik.reshape(NB, 128, 64).transpose(1, 0, 2).reshape(128, NB * 64)
            vvkn_p = np.concatenate([vv_p.reshape(128, NB * 65), sgkn_p], axis=1)

            qkts.append(qkt_p.astype(ml_dtypes.bfloat16))
            sgs.append(sg_p.astype(ml_dtypes.bfloat16))
            vvkns.append(vvkn_p.astype(ml_dtypes.bfloat16))

        in_maps.append({
            "qkt": np.ascontiguousarray(np.stack(qkts)),
            "sg": np.ascontiguousarray(np.stack(sgs)),
            "vvkn": np.ascontiguousarray(np.stack(vvkns)),
            "m01d": np.ascontiguousarray(m01.astype(ml_dtypes.bfloat16)),
        })
    return in_maps


def assemble(results, blend):
    """Normalize + blend on host from raw numerators/denominators."""
    b = float(np.asarray(blend).reshape(-1)[0])
    full = np.empty((N, H, L, E), np.float32)
    for core in range(NCORES):
        r = results[core]
        svlv = np.asarray(r["svlv"], dtype=np.float32)   # [P, 65, NB, 384]
        tails = np.asarray(r["tails"])                   # [P, 128, 2*NB]
        for p in range(PAIRS_PER_CORE):
            g = core * PAIRS_PER_CORE + p
            n, h = g // H, g % H
            sv = svlv[p, :, :, 0:256]       # [65, block, 256]
            lv = svlv[p, :, :, 256:384]     # [65, chunk, 128]
            tl_sum = tails[p, :, 0:NB] + tails[p, :, NB:2 * NB]
            den = tl_sum.T + sv[64, :, 0:128]            # [NB, 128]
            num = sv[0:64, :, 0:128].copy()              # [64, NB, 128]
            num[:, 1:, :] += sv[0:64, 0:NB - 1, 128:256]
            lvn = lv[0:64]                               # [64, NB, 128]
            lvd = lv[64]                                 # [NB, 128]
            o = (b * num / den[None] +
                 (1.0 - b) * lvn / (lvd[None] + EPS))    # [64, NB, 128]
            full[n, h] = o.transpose(1, 2, 0).reshape(L, E)
    return np.ascontiguousarray(np.transpose(full, (0, 2, 1, 3)))


def kernel(queries, keys, values, key_lengths_mask, blend, _trace=False):
    if "nc" not in _cached:
        _cached["nc"] = build_nc()
    nc = _cached["nc"]
    in_maps = host_prep(queries, keys, values, key_lengths_mask, blend)
    res = bass_utils.run_bass_kernel_spmd(nc, in_maps, core_ids=list(range(NCORES)),
                                          trace=_trace)
    _cached["last_results"] = res
    return assemble(res.results, blend)



# revision 4
# speedup vs baseline: 1.0068x; 1.0068x over previous
"""Trainium2 Bass kernel for nn_CausalRankKAttention.

Blend of banded-softmax attention and cumsum linear attention, per (n,h) pair.
16 pairs sharded over 8 NeuronCores (2 pairs/core), no cross-core comm.

Design (v7, evolved from v6):
  - feature map phi(x)=tanh(x)+1 on HOST; only the exp table ever loads on ACT.
  - all matmuls bf16 (PE streams ~1 col/cycle; fp8 DoubleRow measured no
    faster in this environment).
  - per iteration n (ascending):
      band: transposed tile st[s=n, q in n..n+2) -> exp -> mask.
      tail for block n+1 (one iteration EARLY, so nothing big trails at the
        end): forward scores q-block n+1 vs s-blocks [0, n+1) -> wide psum
        units [128,<=1024], one ACT exp per unit with fused accum_out.
      linear: sg scores -> causal mask -> kn state delta.
      trailing output matmuls for block n-1 (vv stationary shared between
        softmax and linear paths; state-apply accumulates into the same psum).
  - v7 changes vs v6:
      * input DMAs staged fine-grained across SP/ACT/Pool DGEs in
        consumption order -> compute starts ~3us instead of ~11us.
      * tails run one block ahead (no hoisted block-15 burst, no tail-end
        serial drain).
      * per-MM pair interleaving so consecutive PE matmuls never target the
        same PSUM bank (removes ~100-170ns WAR/WAW stalls on small matmuls).
      * dummy exp at t=0 pulls the 1.28us ACT table load into the DMA window.
      * small PE warmup burst during the DMA window (HAM clock probe).
      * vkn dram layout [128, NB, 129] so block-range input DMA is contiguous.
  - outputs are RAW numerators/denominators; normalize + blend on host.
"""

import numpy as np
import ml_dtypes

import concourse.bass as bass
import concourse.bacc as bacc
import concourse.mybir as mybir
import concourse.tile as tile
from concourse import bass_utils

F32 = mybir.dt.float32
BF16 = mybir.dt.bfloat16
AF = mybir.ActivationFunctionType
OP = mybir.AluOpType

N, L, H, E = 2, 2048, 8, 64
NB = L // 128            # 16 blocks/chunks of 128
TEMP = float(1.0 / np.sqrt(E))
EPS = 1e-6
PAIRS_PER_CORE = 2
NCORES = 8

_cached = {}


def build_nc():
    nc = bacc.Bacc("TRN2", target_bir_lowering=False, debug=False,
                   num_devices=NCORES)
    P = PAIRS_PER_CORE
    # ---- dram tensors (per core) ----
    # qkt[p, :, 0] = kt (k^T + gate ext row), [p, :, 1] = qt (q^T + ones row)
    qkt = nc.dram_tensor("qkt", [P, 65, 2, L], BF16, kind="ExternalInput")
    sg = nc.dram_tensor("sg", [P, 64, 2, L], BF16, kind="ExternalInput")
    # vkn[:, i, 0:65] = [v*kl | kl] chunk i ; [:, i, 65:129] = phik chunk i
    vkn = nc.dram_tensor("vkn", [P, 128, NB, 129], BF16, kind="ExternalInput")
    m01d = nc.dram_tensor("m01d", [128, 256], BF16, kind="ExternalInput")
    svlv = nc.dram_tensor("svlv", [P, 65, NB, 384], BF16, kind="ExternalOutput")
    # two accumulator columns per block (tail split in halves); host sums
    tails = nc.dram_tensor("tails", [P, 128, 2 * NB], F32, kind="ExternalOutput")

    with tile.TileContext(nc) as tc:
        with (
            tc.tile_pool(name="const", bufs=1) as constp,
            tc.tile_pool(name="io", bufs=1) as iop,
            tc.tile_pool(name="acc", bufs=1) as accp,
            tc.tile_pool(name="work", bufs=2) as workp,
            tc.tile_pool(name="sp", bufs=3) as sp,
            tc.tile_pool(name="tailp", bufs=2, space="PSUM") as tailp,
            tc.tile_pool(name="scp", bufs=2, space="PSUM") as scp,
            tc.tile_pool(name="otp", bufs=2, space="PSUM") as otp,
        ):
            m01_sb = constp.tile([128, 256], BF16, tag="m01")
            scratch = constp.tile([128, 8], F32, tag="scr0")

            # ---- ACT exp-table preload during the DMA window ----
            nc.gpsimd.memset(scratch[:], 0.0)
            nc.scalar.activation(scratch[:, 4:8], scratch[:, 0:4], AF.Exp)

            # ---- input tiles ----
            qkt_sbs, sg_sbs, vkn_sbs, accs, tacc = [], [], [], [], []
            for p in range(P):
                qkt_sb = iop.tile([65, 2, L], BF16, tag=f"qkt{p}")
                sg_sb = iop.tile([64, 2, L], BF16, tag=f"sg{p}")
                vkn_sb = iop.tile([128, NB, 129], BF16, tag=f"vkn{p}")
                qkt_sbs.append(qkt_sb)
                sg_sbs.append(sg_sb)
                vkn_sbs.append(vkn_sb)
                a = accp.tile([65, NB, 384], BF16, tag=f"acc{p}")
                t = accp.tile([128, 2 * NB], F32, tag=f"tails{p}")
                nc.gpsimd.memset(t[:], 0.0)
                accs.append(a)
                tacc.append(t)

            # ---- staged input DMAs, consumption order, 3 DGEs in parallel.
            # SP: m01 + qkt. ACT: vkn. Pool(SWDGE): sg.
            nc.sync.dma_start(m01_sb[:], m01d[:])
            for p in range(P):
                nc.sync.dma_start(qkt_sbs[p][:, :, 0:512], qkt[p, :, :, 0:512])
            for p in range(P):
                nc.gpsimd.dma_start(sg_sbs[p][:, :, 0:512], sg[p, :, :, 0:512])
            for p in range(P):
                nc.scalar.dma_start(vkn_sbs[p][:, 0:4, :], vkn[p, :, 0:4, :])
            for p in range(P):
                nc.sync.dma_start(qkt_sbs[p][:, :, 512:1024],
                                  qkt[p, :, :, 512:1024])
            for p in range(P):
                nc.gpsimd.dma_start(sg_sbs[p][:, :, 512:2048],
                                    sg[p, :, :, 512:2048])
            for p in range(P):
                nc.scalar.dma_start(vkn_sbs[p][:, 4:16, :], vkn[p, :, 4:16, :])
            for p in range(P):
                nc.sync.dma_start(qkt_sbs[p][:, :, 1024:2048],
                                  qkt[p, :, :, 1024:2048])

            # ---- PE warmup probe during the DMA window (HAM clock) ----
            warm = otp.tile([65, 384], F32, tag="ot")
            for i in range(4):
                lo = 0 if i % 2 == 0 else 192
                nc.tensor.matmul(warm[:, lo:lo + 192], m01_sb[:, 0:65],
                                 m01_sb[:, 0:192], start=True, stop=True,
                                 skip_group_check=True)

            def kt(p):
                return qkt_sbs[p][:, 0, :]

            def qt(p):
                return qkt_sbs[p][:, 1, :]

            def sgk(p):
                return sg_sbs[p][:, 0, :]

            def sgq(p):
                return sg_sbs[p][:, 1, :]

            def vv_ap(p, i):
                return vkn_sbs[p][:, i, 0:65]

            def sgkn_ap(p, c):
                return vkn_sbs[p][:, c, 65:129]

            s_cur = [None, None]
            prev = [None, None]
            for n in range(NB):
                qw = 256 if n < NB - 1 else 128
                c0, c1 = n * 128, (n + 1) * 128
                tb = n + 1                       # tail block (one ahead)
                tw = tb * 128 if tb <= NB - 1 else 0
                tw1 = min(tw, 1024)

                # -- 1. band score MMs (pair-interleaved) --
                scs = []
                for p in range(P):
                    sc = scp.tile([128, 512], F32, tag="sc")
                    nc.tensor.matmul(sc[:, 0:qw], kt(p)[:, c0:c1],
                                     qt(p)[:, c0:c0 + qw],
                                     start=True, stop=True)
                    scs.append(sc)
                # -- 2. sg score MMs --
                for p in range(P):
                    nc.tensor.matmul(scs[p][:, 256:384], sgk(p)[:, c0:c1],
                                     sgq(p)[:, c0:c1], start=True, stop=True)
                # -- 3. kn state-delta MMs --
                for p in range(P):
                    nc.tensor.matmul(scs[p][0:64, 384:449], sgkn_ap(p, n),
                                     vv_ap(p, n), start=True, stop=True)
                # -- 4. tail unit-1 MMs for block tb --
                tps1 = [None, None]
                if tw:
                    for p in range(P):
                        tp_ = tailp.tile([128, 1024], F32, tag="tp")
                        for off in range(0, tw1, 512):
                            n_ = min(512, tw1 - off)
                            nc.tensor.matmul(tp_[:, off:off + n_],
                                             qt(p)[:, tb * 128:tb * 128 + 128],
                                             kt(p)[:, off:off + n_],
                                             start=True, stop=True)
                        tps1[p] = tp_
                # -- 5. trailing output MMs for block n-1 --
                if prev[0] is not None:
                    pn = n - 1
                    for p in range(P):
                        pot = prev[p]["ot"]
                        nc.tensor.matmul(pot[:], vv_ap(p, pn),
                                         prev[p]["stat"][:], start=True,
                                         stop=(pn == 0), skip_group_check=True)
                    if pn > 0:
                        for p in range(P):
                            pot = prev[p]["ot"]
                            nc.tensor.matmul(pot[:, 256:384],
                                             prev[p]["s_before"][:],
                                             sgq(p)[:, pn * 128:pn * 128 + 128],
                                             start=False, stop=True,
                                             skip_group_check=True)
                # -- 6. ACT: band exps then tail unit-1 exps --
                st_es = []
                for p in range(P):
                    st_e = workp.tile([128, 256], BF16, tag="st_e")
                    nc.scalar.activation(st_e[:, 0:qw], scs[p][:, 0:qw],
                                         AF.Exp, scale=TEMP)
                    st_es.append(st_e)
                if tw:
                    for p in range(P):
                        scr = workp.tile([128, 1024], BF16, tag="scr")
                        nc.scalar.activation(scr[:, 0:tw1], tps1[p][:, 0:tw1],
                                             AF.Exp, scale=TEMP,
                                             accum_out=tacc[p][:, tb:tb + 1])
                # -- 7. tail unit-2 MMs + exps (wide blocks) --
                if tw > 1024:
                    tps2 = []
                    for p in range(P):
                        tp_ = tailp.tile([128, 1024], F32, tag="tp")
                        for off in range(1024, tw, 512):
                            n_ = min(512, tw - off)
                            nc.tensor.matmul(tp_[:, off - 1024:off - 1024 + n_],
                                             qt(p)[:, tb * 128:tb * 128 + 128],
                                             kt(p)[:, off:off + n_],
                                             start=True, stop=True)
                        tps2.append(tp_)
                    for p in range(P):
                        scr = workp.tile([128, 1024], BF16, tag="scr")
                        nc.scalar.activation(scr[:, 0:tw - 1024],
                                             tps2[p][:, 0:tw - 1024],
                                             AF.Exp, scale=TEMP,
                                             accum_out=tacc[p][:, NB + tb:NB + tb + 1])
                # -- 8. DVE: masks + state update, then drain block n-1 --
                stats = []
                for p in range(P):
                    stat = workp.tile([128, 384], BF16, tag="stat")
                    nc.vector.tensor_tensor(stat[:, 0:qw], st_es[p][:, 0:qw],
                                            m01_sb[:, 0:qw], OP.mult)
                    if qw < 256:
                        nc.vector.memset(stat[:, 128:256], 0.0)
                    nc.vector.tensor_tensor(stat[:, 256:384],
                                            scs[p][:, 256:384],
                                            m01_sb[:, 0:128], OP.mult)
                    stats.append(stat)
                s_before = [s_cur[0], s_cur[1]]
                for p in range(P):
                    s_nxt = sp.tile([64, 65], BF16, tag=f"s{p}")
                    if n == 0:
                        nc.vector.tensor_copy(s_nxt[:], scs[p][0:64, 384:449])
                    else:
                        nc.vector.scalar_tensor_tensor(s_nxt[:], s_cur[p][:],
                                                       1.0,
                                                       scs[p][0:64, 384:449],
                                                       OP.mult, OP.add)
                    s_cur[p] = s_nxt
                if prev[0] is not None:
                    for p in range(P):
                        nc.vector.tensor_copy(accs[p][:, n - 1, :],
                                              prev[p]["ot"][:])
                # -- 9. bookkeeping + periodic output drain --
                for p in range(P):
                    ot = otp.tile([65, 384], F32, tag="ot")
                    prev[p] = {"ot": ot, "stat": stats[p],
                               "s_before": s_before[p]}
                if n in (5, 9, 13):
                    for p in range(P):
                        nc.gpsimd.dma_start(svlv[p, :, n - 5:n - 1, :],
                                            accs[p][:, n - 5:n - 1, :])

            # ---- epilogue: finish last block for both pairs ----
            pn = NB - 1
            for p in range(P):
                pot = prev[p]["ot"]
                nc.tensor.matmul(pot[:], vv_ap(p, pn), prev[p]["stat"][:],
                                 start=True, stop=False, skip_group_check=True)
            for p in range(P):
                pot = prev[p]["ot"]
                nc.tensor.matmul(pot[:, 256:384], prev[p]["s_before"][:],
                                 sgq(p)[:, pn * 128:pn * 128 + 128],
                                 start=False, stop=True, skip_group_check=True)
            for p in range(P):
                nc.vector.tensor_copy(accs[p][:, pn, :], prev[p]["ot"][:])
            for p in range(P):
                nc.scalar.dma_start(svlv[p, :, 12:NB, :], accs[p][:, 12:NB, :])
                nc.sync.dma_start(tails[p], tacc[p][:])

    nc.compile()
    return nc


def host_prep(queries, keys, values, key_lengths_mask, blend):
    """Build per-core in_maps from full inputs."""
    q = np.ascontiguousarray(np.transpose(queries, (0, 2, 1, 3)))  # [N,H,L,E]
    k = np.ascontiguousarray(np.transpose(keys, (0, 2, 1, 3)))
    v = np.ascontiguousarray(np.transpose(values, (0, 2, 1, 3)))
    q = q.reshape(N * H, L, E).astype(np.float32)
    k = k.reshape(N * H, L, E).astype(np.float32)
    v = v.reshape(N * H, L, E).astype(np.float32)
    klm = np.asarray(key_lengths_mask, np.float32)  # [N, L]

    ii = np.arange(128)[:, None]
    cc = np.arange(256)[None, :]
    m01 = ((cc - ii >= 0) & (cc - ii <= 128)).astype(np.float32)

    in_maps = []
    for core in range(NCORES):
        qkts, sgs, vkns = [], [], []
        for p in range(PAIRS_PER_CORE):
            g = core * PAIRS_PER_CORE + p
            n = g // H
            qg, kg, vg = q[g], k[g], v[g]          # [L, E]
            kl = klm[n]                             # [L]
            i01 = (kl > 0).astype(np.float32)

            qkt_p = np.empty((65, 2, L), np.float32)
            qkt_p[0:64, 0] = kg.T
            qkt_p[64, 0] = -1e9 * (1.0 - i01)
            qkt_p[0:64, 1] = qg.T
            qkt_p[64, 1] = 1.0

            phiq = np.tanh(qg) + 1.0
            phik = np.tanh(kg) + 1.0
            sg_p = np.empty((64, 2, L), np.float32)
            sg_p[:, 0] = phik.T
            sg_p[:, 1] = phiq.T

            vv_full = np.empty((L, 65), np.float32)
            vv_full[:, 0:64] = vg * kl[:, None]
            vv_full[:, 64] = kl
            vkn_p = np.empty((128, NB, 129), np.float32)
            vkn_p[:, :, 0:65] = vv_full.reshape(NB, 128, 65).transpose(1, 0, 2)
            vkn_p[:, :, 65:129] = phik.reshape(NB, 128, 64).transpose(1, 0, 2)

            qkts.append(qkt_p.astype(ml_dtypes.bfloat16))
            sgs.append(sg_p.astype(ml_dtypes.bfloat16))
            vkns.append(vkn_p.astype(ml_dtypes.bfloat16))

        in_maps.append({
            "qkt": np.ascontiguousarray(np.stack(qkts)),
            "sg": np.ascontiguousarray(np.stack(sgs)),
            "vkn": np.ascontiguousarray(np.stack(vkns)),
            "m01d": np.ascontiguousarray(m01.astype(ml_dtypes.bfloat16)),
        })
    return in_maps


def assemble(results, blend):
    """Normalize + blend on host from raw numerators/denominators."""
    b = float(np.asarray(blend).reshape(-1)[0])
    full = np.empty((N, H, L, E), np.float32)
    for core in range(NCORES):
        r = results[core]
        svlv = np.asarray(r["svlv"], dtype=np.float32)   # [P, 65, NB, 384]
        tails = np.asarray(r["tails"])                   # [P, 128, 2*NB]
        for p in range(PAIRS_PER_CORE):
            g = core * PAIRS_PER_CORE + p
            n, h = g // H, g % H
            sv = svlv[p, :, :, 0:256]       # [65, block, 256]
            lv = svlv[p, :, :, 256:384]     # [65, chunk, 128]
            tl_sum = tails[p, :, 0:NB] + tails[p, :, NB:2 * NB]
            den = tl_sum.T + sv[64, :, 0:128]            # [NB, 128]
            num = sv[0:64, :, 0:128].copy()              # [64, NB, 128]
            num[:, 1:, :] += sv[0:64, 0:NB - 1, 128:256]
            lvn = lv[0:64]                               # [64, NB, 128]
            lvd = lv[64]                                 # [NB, 128]
            o = (b * num / den[None] +
                 (1.0 - b) * lvn / (lvd[None] + EPS))    # [64, NB, 128]
            full[n, h] = o.transpose(1, 2, 0).reshape(L, E)
    return np.ascontiguousarray(np.transpose(full, (0, 2, 1, 3)))


def kernel(queries, keys, values, key_lengths_mask, blend, _trace=False):
    if "nc" not in _cached:
        _cached["nc"] = build_nc()
    nc = _cached["nc"]
    in_maps = host_prep(queries, keys, values, key_lengths_mask, blend)
    res = bass_utils.run_bass_kernel_spmd(nc, in_maps, core_ids=list(range(NCORES)),
                                          trace=_trace)
    _cached["last_results"] = res
    return assemble(res.results, blend)


# revision 5
# speedup vs baseline: 1.0484x; 1.0413x over previous
"""Trainium2 Bass kernel for nn_CausalRankKAttention.

Blend of banded-softmax attention and cumsum linear attention, per (n,h) pair.
16 pairs sharded over 8 NeuronCores (2 pairs/core), no cross-core comm.

Design (v8):
  - feature map phi(x)=tanh(x)+1 on HOST; only the exp table ever loads on ACT.
  - all matmuls bf16. Microbenchmarked PE facts for this environment: column
    rate is 1 col/cycle at 1.2GHz regardless of dtype (fp8 DoubleRow included),
    the clock never ramps to 2.4GHz, LDWEIGHTS is fully hidden, and
    stationary/bank switches are free. So PE floor = total moving columns, and
    ACT (exp) is the co-dominant engine: minimize ACT columns + instruction
    count, then keep both engines decoupled.
  - per iteration n (ascending):
      band: transposed tile st[s=n, q in n..n+2) for BOTH pairs into one psum
        bank -> ONE merged exp -> per-pair masks on DVE.
      tail blocks run one-or-more iterations EARLY (blocks 13-15's units are
        spread over iters 12-14) so the ACT queue drains before the epilogue.
      linear: sg scores + kn state delta for both pairs share one psum bank.
      trailing output matmuls for block n-1 (vv stationary shared between
        softmax and linear paths; state-apply accumulates into the same psum).
  - input DMAs staged fine-grained across SP/ACT/Pool DGEs in consumption
    order; first compute needs only qkt cols 0:256.
  - outputs are RAW numerators/denominators; normalize + blend on host.
"""

import numpy as np
import ml_dtypes

import concourse.bass as bass
import concourse.bacc as bacc
import concourse.mybir as mybir
import concourse.tile as tile
from concourse import bass_utils

F32 = mybir.dt.float32
BF16 = mybir.dt.bfloat16
AF = mybir.ActivationFunctionType
OP = mybir.AluOpType

N, L, H, E = 2, 2048, 8, 64
NB = L // 128            # 16 blocks/chunks of 128
TEMP = float(1.0 / np.sqrt(E))
EPS = 1e-6
PAIRS_PER_CORE = 2
NCORES = 8

_cached = {}

# tail work distribution: iteration -> list of (block, unit). unit1 covers
# s in [0, min(w,1024)), unit2 covers [1024, w). Last blocks pulled early so
# the ACT queue is empty by the epilogue.
TAIL_SCHED = {n: [(n + 1, 1)] + ([(n + 1, 2)] if (n + 1) * 128 > 1024 else [])
              for n in range(12)}
TAIL_SCHED[12] = [(13, 1), (13, 2), (14, 1)]
TAIL_SCHED[13] = [(14, 2), (15, 1)]
TAIL_SCHED[14] = [(15, 2)]
TAIL_SCHED[15] = []


def build_nc():
    nc = bacc.Bacc("TRN2", target_bir_lowering=False, debug=False,
                   num_devices=NCORES)
    P = PAIRS_PER_CORE
    # ---- dram tensors (per core) ----
    # qkt[p, :, 0] = kt (k^T + gate ext row), [p, :, 1] = qt (q^T + ones row)
    qkt = nc.dram_tensor("qkt", [P, 65, 2, L], BF16, kind="ExternalInput")
    sg = nc.dram_tensor("sg", [P, 64, 2, L], BF16, kind="ExternalInput")
    # vkn[:, i, 0:65] = [v*kl | kl] chunk i ; [:, i, 65:129] = phik chunk i
    vkn = nc.dram_tensor("vkn", [P, 128, NB, 129], BF16, kind="ExternalInput")
    m01d = nc.dram_tensor("m01d", [128, 256], BF16, kind="ExternalInput")
    svlv = nc.dram_tensor("svlv", [P, 65, NB, 384], BF16, kind="ExternalOutput")
    # two accumulator columns per block (tail split in halves); host sums
    tails = nc.dram_tensor("tails", [P, 128, 2 * NB], F32, kind="ExternalOutput")

    with tile.TileContext(nc) as tc:
        with (
            tc.tile_pool(name="const", bufs=1) as constp,
            tc.tile_pool(name="io", bufs=1) as iop,
            tc.tile_pool(name="acc", bufs=1) as accp,
            tc.tile_pool(name="work", bufs=3) as workp,
            tc.tile_pool(name="sp", bufs=3) as sp,
            tc.tile_pool(name="tailp", bufs=2, space="PSUM") as tailp,
            tc.tile_pool(name="bandp", bufs=1, space="PSUM") as bandp,
            tc.tile_pool(name="sknp", bufs=1, space="PSUM") as sknp,
            tc.tile_pool(name="otp", bufs=2, space="PSUM") as otp,
        ):
            m01_sb = constp.tile([128, 256], BF16, tag="m01")
            scratch = constp.tile([128, 8], F32, tag="scr0")

            # ---- ACT exp-table preload during the DMA window ----
            nc.gpsimd.memset(scratch[:], 0.0)
            nc.scalar.activation(scratch[:, 4:8], scratch[:, 0:4], AF.Exp)

            # ---- input tiles ----
            qkt_sbs, sg_sbs, vkn_sbs, accs, tacc = [], [], [], [], []
            for p in range(P):
                qkt_sb = iop.tile([65, 2, L], BF16, tag=f"qkt{p}")
                sg_sb = iop.tile([64, 2, L], BF16, tag=f"sg{p}")
                vkn_sb = iop.tile([128, NB, 129], BF16, tag=f"vkn{p}")
                qkt_sbs.append(qkt_sb)
                sg_sbs.append(sg_sb)
                vkn_sbs.append(vkn_sb)
                a = accp.tile([65, NB, 384], BF16, tag=f"acc{p}")
                t = accp.tile([128, 2 * NB], F32, tag=f"tails{p}")
                nc.gpsimd.memset(t[:], 0.0)
                accs.append(a)
                tacc.append(t)

            # ---- staged input DMAs, consumption order, 3 DGEs in parallel.
            # SP: qkt + m01. Pool(SWDGE): sg. ACT: vkn (after table preload).
            for p in range(P):
                nc.sync.dma_start(qkt_sbs[p][:, :, 0:256], qkt[p, :, :, 0:256])
            nc.sync.dma_start(m01_sb[:], m01d[:])
            for p in range(P):
                nc.gpsimd.dma_start(sg_sbs[p][:, :, 0:512], sg[p, :, :, 0:512])
            for p in range(P):
                nc.scalar.dma_start(vkn_sbs[p][:, 0:4, :], vkn[p, :, 0:4, :])
            for p in range(P):
                nc.sync.dma_start(qkt_sbs[p][:, :, 256:1024],
                                  qkt[p, :, :, 256:1024])
            for p in range(P):
                nc.gpsimd.dma_start(sg_sbs[p][:, :, 512:2048],
                                    sg[p, :, :, 512:2048])
            for p in range(P):
                nc.scalar.dma_start(vkn_sbs[p][:, 4:16, :], vkn[p, :, 4:16, :])
            for p in range(P):
                nc.sync.dma_start(qkt_sbs[p][:, :, 1024:2048],
                                  qkt[p, :, :, 1024:2048])

            def kt(p):
                return qkt_sbs[p][:, 0, :]

            def qt(p):
                return qkt_sbs[p][:, 1, :]

            def sgk(p):
                return sg_sbs[p][:, 0, :]

            def sgq(p):
                return sg_sbs[p][:, 1, :]

            def vv_ap(p, i):
                return vkn_sbs[p][:, i, 0:65]

            def sgkn_ap(p, c):
                return vkn_sbs[p][:, c, 65:129]

            s_cur = [None, None]
            prev = [None, None]
            for n in range(NB):
                qw = 256 if n < NB - 1 else 128
                c0, c1 = n * 128, (n + 1) * 128
                units = TAIL_SCHED[n]

                # -- 1. band score MMs, both pairs into one bank --
                bandt = bandp.tile([128, 512], F32, tag="bandt")
                for p in range(P):
                    nc.tensor.matmul(bandt[:, 256 * p:256 * p + qw],
                                     kt(p)[:, c0:c1], qt(p)[:, c0:c0 + qw],
                                     start=True, stop=True,
                                     skip_group_check=True)
                # -- 2. sg + kn MMs, both pairs into one bank --
                sknt = sknp.tile([128, 512], F32, tag="sknt")
                for p in range(P):
                    nc.tensor.matmul(sknt[:, 128 * p:128 * (p + 1)],
                                     sgk(p)[:, c0:c1], sgq(p)[:, c0:c1],
                                     start=True, stop=True,
                                     skip_group_check=True)
                for p in range(P):
                    nc.tensor.matmul(sknt[0:64, 256 + 65 * p:321 + 65 * p],
                                     sgkn_ap(p, n), vv_ap(p, n),
                                     start=True, stop=True,
                                     skip_group_check=True)
                # -- 3. first tail unit MMs --
                tptiles = []
                for tb, unit in units:
                    lo = 0 if unit == 1 else 1024
                    hi = min(tb * 128, 1024) if unit == 1 else tb * 128
                    for p in range(P):
                        tp_ = tailp.tile([128, 1024], F32, tag="tp")
                        for off in range(lo, hi, 512):
                            n_ = min(512, hi - off)
                            nc.tensor.matmul(tp_[:, off - lo:off - lo + n_],
                                             qt(p)[:, tb * 128:tb * 128 + 128],
                                             kt(p)[:, off:off + n_],
                                             start=True, stop=True)
                        tptiles.append((tb, unit, p, tp_, hi - lo))
                    if unit == 1 and len(units) > 1:
                        break   # issue remaining units after the out MMs
                # -- 4. trailing output MMs for block n-1 --
                if prev[0] is not None:
                    pn = n - 1
                    for p in range(P):
                        pot = prev[p]["ot"]
                        nc.tensor.matmul(pot[:], vv_ap(p, pn),
                                         prev[p]["stat"][:], start=True,
                                         stop=(pn == 0), skip_group_check=True)
                    if pn > 0:
                        for p in range(P):
                            pot = prev[p]["ot"]
                            nc.tensor.matmul(pot[:, 256:384],
                                             prev[p]["s_before"][:],
                                             sgq(p)[:, pn * 128:pn * 128 + 128],
                                             start=False, stop=True,
                                             skip_group_check=True)
                # -- 5. remaining tail unit MMs --
                done_units = {(tb, u) for tb, u, _, _, _ in tptiles}
                for tb, unit in units:
                    if (tb, unit) in done_units:
                        continue
                    lo = 0 if unit == 1 else 1024
                    hi = min(tb * 128, 1024) if unit == 1 else tb * 128
                    for p in range(P):
                        tp_ = tailp.tile([128, 1024], F32, tag="tp")
                        for off in range(lo, hi, 512):
                            n_ = min(512, hi - off)
                            nc.tensor.matmul(tp_[:, off - lo:off - lo + n_],
                                             qt(p)[:, tb * 128:tb * 128 + 128],
                                             kt(p)[:, off:off + n_],
                                             start=True, stop=True)
                        tptiles.append((tb, unit, p, tp_, hi - lo))
                # -- 6. ACT: ONE merged band exp, then tail exps --
                st_e = workp.tile([128, 512], BF16, tag="st_e")
                nc.scalar.activation(st_e[:, 0:256 + qw], bandt[:, 0:256 + qw],
                                     AF.Exp, scale=TEMP)
                for tb, unit, p, tp_, w_ in tptiles:
                    acccol = tb if unit == 1 else NB + tb
                    scr = workp.tile([128, 1024], BF16, tag="scr")
                    nc.scalar.activation(scr[:, 0:w_], tp_[:, 0:w_],
                                         AF.Exp, scale=TEMP,
                                         accum_out=tacc[p][:, acccol:acccol + 1])
                # -- 7. DVE: masks + state update, then drain block n-1 --
                stats = []
                for p in range(P):
                    stat = workp.tile([128, 384], BF16, tag="stat")
                    nc.vector.tensor_tensor(stat[:, 0:qw],
                                            st_e[:, 256 * p:256 * p + qw],
                                            m01_sb[:, 0:qw], OP.mult)
                    if qw < 256:
                        nc.vector.memset(stat[:, 128:256], 0.0)
                    nc.vector.tensor_tensor(stat[:, 256:384],
                                            sknt[:, 128 * p:128 * (p + 1)],
                                            m01_sb[:, 0:128], OP.mult)
                    stats.append(stat)
                s_before = [s_cur[0], s_cur[1]]
                for p in range(P):
                    s_nxt = sp.tile([64, 65], BF16, tag=f"s{p}")
                    kn_ap = sknt[0:64, 256 + 65 * p:321 + 65 * p]
                    if n == 0:
                        nc.vector.tensor_copy(s_nxt[:], kn_ap)
                    else:
                        nc.vector.scalar_tensor_tensor(s_nxt[:], s_cur[p][:],
                                                       1.0, kn_ap,
                                                       OP.mult, OP.add)
                    s_cur[p] = s_nxt
                if prev[0] is not None:
                    for p in range(P):
                        nc.vector.tensor_copy(accs[p][:, n - 1, :],
                                              prev[p]["ot"][:])
                # -- 8. bookkeeping + periodic output drain --
                for p in range(P):
                    ot = otp.tile([65, 384], F32, tag="ot")
                    prev[p] = {"ot": ot, "stat": stats[p],
                               "s_before": s_before[p]}
                if n in (5, 9, 13):
                    for p in range(P):
                        nc.gpsimd.dma_start(svlv[p, :, n - 5:n - 1, :],
                                            accs[p][:, n - 5:n - 1, :])
                if n == NB - 1:
                    # all tail accums are complete (last unit ran at iter 14)
                    for p in range(P):
                        nc.sync.dma_start(tails[p], tacc[p][:])
                        nc.gpsimd.dma_start(svlv[p, :, 12:15, :],
                                            accs[p][:, 12:15, :])

            # ---- epilogue: finish last block for both pairs ----
            pn = NB - 1
            for p in range(P):
                pot = prev[p]["ot"]
                nc.tensor.matmul(pot[:], vv_ap(p, pn), prev[p]["stat"][:],
                                 start=True, stop=False, skip_group_check=True)
            for p in range(P):
                pot = prev[p]["ot"]
                nc.tensor.matmul(pot[:, 256:384], prev[p]["s_before"][:],
                                 sgq(p)[:, pn * 128:pn * 128 + 128],
                                 start=False, stop=True, skip_group_check=True)
            for p in range(P):
                nc.vector.tensor_copy(accs[p][:, pn, :], prev[p]["ot"][:])
            for p in range(P):
                nc.scalar.dma_start(svlv[p, :, 15:NB, :], accs[p][:, 15:NB, :])

    nc.compile()
    return nc


def host_prep(queries, keys, values, key_lengths_mask, blend):
    """Build per-core in_maps from full inputs."""
    q = np.ascontiguousarray(np.transpose(queries, (0, 2, 1, 3)))  # [N,H,L,E]
    k = np.ascontiguousarray(np.transpose(keys, (0, 2, 1, 3)))
    v = np.ascontiguousarray(np.transpose(values, (0, 2, 1, 3)))
    q = q.reshape(N * H, L, E).astype(np.float32)
    k = k.reshape(N * H, L, E).astype(np.float32)
    v = v.reshape(N * H, L, E).astype(np.float32)
    klm = np.asarray(key_lengths_mask, np.float32)  # [N, L]

    ii = np.arange(128)[:, None]
    cc = np.arange(256)[None, :]
    m01 = ((cc - ii >= 0) & (cc - ii <= 128)).astype(np.float32)

    in_maps = []
    for core in range(NCORES):
        qkts, sgs, vkns = [], [], []
        for p in range(PAIRS_PER_CORE):
            g = core * PAIRS_PER_CORE + p
            n = g // H
            qg, kg, vg = q[g], k[g], v[g]          # [L, E]
            kl = klm[n]                             # [L]
            i01 = (kl > 0).astype(np.float32)

            qkt_p = np.empty((65, 2, L), np.float32)
            qkt_p[0:64, 0] = kg.T
            qkt_p[64, 0] = -1e9 * (1.0 - i01)
            qkt_p[0:64, 1] = qg.T
            qkt_p[64, 1] = 1.0

            phiq = np.tanh(qg) + 1.0
            phik = np.tanh(kg) + 1.0
            sg_p = np.empty((64, 2, L), np.float32)
            sg_p[:, 0] = phik.T
            sg_p[:, 1] = phiq.T

            vv_full = np.empty((L, 65), np.float32)
            vv_full[:, 0:64] = vg * kl[:, None]
            vv_full[:, 64] = kl
            vkn_p = np.empty((128, NB, 129), np.float32)
            vkn_p[:, :, 0:65] = vv_full.reshape(NB, 128, 65).transpose(1, 0, 2)
            vkn_p[:, :, 65:129] = phik.reshape(NB, 128, 64).transpose(1, 0, 2)

            qkts.append(qkt_p.astype(ml_dtypes.bfloat16))
            sgs.append(sg_p.astype(ml_dtypes.bfloat16))
            vkns.append(vkn_p.astype(ml_dtypes.bfloat16))

        in_maps.append({
            "qkt": np.ascontiguousarray(np.stack(qkts)),
            "sg": np.ascontiguousarray(np.stack(sgs)),
            "vkn": np.ascontiguousarray(np.stack(vkns)),
            "m01d": np.ascontiguousarray(m01.astype(ml_dtypes.bfloat16)),
        })
    return in_maps


def assemble(results, blend):
    """Normalize + blend on host from raw numerators/denominators."""
    b = float(np.asarray(blend).reshape(-1)[0])
    full = np.empty((N, H, L, E), np.float32)
    for core in range(NCORES):
        r = results[core]
        svlv = np.asarray(r["svlv"], dtype=np.float32)   # [P, 65, NB, 384]
        tails = np.asarray(r["tails"])                   # [P, 128, 2*NB]
        for p in range(PAIRS_PER_CORE):
            g = core * PAIRS_PER_CORE + p
            n, h = g // H, g % H
            sv = svlv[p, :, :, 0:256]       # [65, block, 256]
            lv = svlv[p, :, :, 256:384]     # [65, chunk, 128]
            tl_sum = tails[p, :, 0:NB] + tails[p, :, NB:2 * NB]
            den = tl_sum.T + sv[64, :, 0:128]            # [NB, 128]
            num = sv[0:64, :, 0:128].copy()              # [64, NB, 128]
            num[:, 1:, :] += sv[0:64, 0:NB - 1, 128:256]
            lvn = lv[0:64]                               # [64, NB, 128]
            lvd = lv[64]                                 # [NB, 128]
            o = (b * num / den[None] +
                 (1.0 - b) * lvn / (lvd[None] + EPS))    # [64, NB, 128]
            full[n, h] = o.transpose(1, 2, 0).reshape(L, E)
    return np.ascontiguousarray(np.transpose(full, (0, 2, 1, 3)))


def kernel(queries, keys, values, key_lengths_mask, blend, _trace=False):
    if "nc" not in _cached:
        _cached["nc"] = build_nc()
    nc = _cached["nc"]
    in_maps = host_prep(queries, keys, values, key_lengths_mask, blend)
    res = bass_utils.run_bass_kernel_spmd(nc, in_maps, core_ids=list(range(NCORES)),
                                          trace=_trace)
    _cached["last_results"] = res
    return assemble(res.results, blend)


# revision 8
# speedup vs baseline: 1.0656x; 1.0164x over previous
"""Trainium2 Bass kernel for nn_CausalRankKAttention.

Blend of banded-softmax attention and cumsum linear attention, per (n,h) pair.
16 pairs sharded over 8 NeuronCores (2 pairs/core), no cross-core comm.

Design (v8):
  - feature map phi(x)=tanh(x)+1 on HOST; only the exp table ever loads on ACT.
  - all matmuls bf16. Microbenchmarked PE facts for this environment: column
    rate is 1 col/cycle at 1.2GHz regardless of dtype (fp8 DoubleRow included),
    the clock never ramps to 2.4GHz, LDWEIGHTS is fully hidden, and
    stationary/bank switches are free. So PE floor = total moving columns, and
    ACT (exp) is the co-dominant engine: minimize ACT columns + instruction
    count, then keep both engines decoupled.
  - per iteration n (ascending):
      band: transposed tile st[s=n, q in n..n+2) for BOTH pairs into one psum
        bank -> ONE merged exp -> per-pair masks on DVE.
      tail blocks run one-or-more iterations EARLY (blocks 13-15's units are
        spread over iters 12-14) so the ACT queue drains before the epilogue.
      linear: sg scores + kn state delta for both pairs share one psum bank.
      trailing output matmuls for block n-1 (vv stationary shared between
        softmax and linear paths; state-apply accumulates into the same psum).
  - input DMAs staged fine-grained across SP/ACT/Pool DGEs in consumption
    order; first compute needs only qkt cols 0:256.
  - outputs are RAW numerators/denominators; normalize + blend on host.
"""

import numpy as np
import ml_dtypes

import concourse.bass as bass
import concourse.bacc as bacc
import concourse.mybir as mybir
import concourse.tile as tile
from concourse import bass_utils

F32 = mybir.dt.float32
BF16 = mybir.dt.bfloat16
AF = mybir.ActivationFunctionType
OP = mybir.AluOpType

N, L, H, E = 2, 2048, 8, 64
NB = L // 128            # 16 blocks/chunks of 128
TEMP = float(1.0 / np.sqrt(E))
EPS = 1e-6
PAIRS_PER_CORE = 2
NCORES = 8

_cached = {}

# tail work distribution: iteration -> list of (block, unit). unit1 covers
# s in [0, min(w,1024)), unit2 covers [1024, w). Last blocks pulled early so
# the ACT queue is empty by the epilogue.
TAIL_SCHED = {n: [(n + 1, 1)] + ([(n + 1, 2)] if (n + 1) * 128 > 1024 else [])
              for n in range(12)}
TAIL_SCHED[12] = [(13, 1), (13, 2), (14, 1)]
TAIL_SCHED[13] = [(14, 2), (15, 1)]
TAIL_SCHED[14] = [(15, 2)]
TAIL_SCHED[15] = []


def build_nc():
    nc = bacc.Bacc("TRN2", target_bir_lowering=False, debug=False,
                   num_devices=NCORES)
    P = PAIRS_PER_CORE
    # ---- dram tensors (per core) ----
    # qkt[p, :, 0] = kt (k^T + gate ext row), [p, :, 1] = qt (q^T + ones row)
    qkt = nc.dram_tensor("qkt", [P, 65, 2, L], BF16, kind="ExternalInput")
    sg = nc.dram_tensor("sg", [P, 64, 2, L], BF16, kind="ExternalInput")
    # vkn[:, i, 0:65] = [v*kl | kl] chunk i ; [:, i, 65:129] = phik chunk i
    vkn = nc.dram_tensor("vkn", [P, 128, NB, 129], BF16, kind="ExternalInput")
    m01d = nc.dram_tensor("m01d", [128, 256], BF16, kind="ExternalInput")
    svlv = nc.dram_tensor("svlv", [P, 65, NB, 384], BF16, kind="ExternalOutput")
    # two accumulator columns per block (tail split in halves); host sums
    tails = nc.dram_tensor("tails", [P, 128, 2 * NB], F32, kind="ExternalOutput")

    with tile.TileContext(nc) as tc:
        with (
            tc.tile_pool(name="const", bufs=1) as constp,
            tc.tile_pool(name="io", bufs=1) as iop,
            tc.tile_pool(name="acc", bufs=1) as accp,
            tc.tile_pool(name="work", bufs=3) as workp,
            tc.tile_pool(name="sp", bufs=3) as sp,
            tc.tile_pool(name="tailp", bufs=2, space="PSUM") as tailp,
            tc.tile_pool(name="bandp", bufs=1, space="PSUM") as bandp,
            tc.tile_pool(name="sknp", bufs=1, space="PSUM") as sknp,
            tc.tile_pool(name="otp", bufs=2, space="PSUM") as otp,
        ):
            m01_sb = constp.tile([128, 256], BF16, tag="m01")
            scratch = constp.tile([128, 8], F32, tag="scr0")

            # ---- ACT exp-table preload during the DMA window ----
            nc.gpsimd.memset(scratch[:], 0.0)
            nc.scalar.activation(scratch[:, 4:8], scratch[:, 0:4], AF.Exp)

            # ---- input tiles ----
            qkt_sbs, sg_sbs, vkn_sbs, accs, tacc = [], [], [], [], []
            for p in range(P):
                qkt_sb = iop.tile([65, 2, L], BF16, tag=f"qkt{p}")
                sg_sb = iop.tile([64, 2, L], BF16, tag=f"sg{p}")
                vkn_sb = iop.tile([128, NB, 129], BF16, tag=f"vkn{p}")
                qkt_sbs.append(qkt_sb)
                sg_sbs.append(sg_sb)
                vkn_sbs.append(vkn_sb)
                a = accp.tile([65, NB, 384], BF16, tag=f"acc{p}")
                t = accp.tile([128, 2 * NB], F32, tag=f"tails{p}")
                nc.gpsimd.memset(t[:], 0.0)
                accs.append(a)
                tacc.append(t)

            # ---- staged input DMAs, consumption order, 3 DGEs in parallel.
            # SP: qkt in 4 stages. Pool(SWDGE): sg in 3. ACT: vkn in 3 + m01.
            # Ordered by earliest consuming iteration so no stream starves.
            for p in range(P):
                nc.sync.dma_start(qkt_sbs[p][:, :, 0:512], qkt[p, :, :, 0:512])
            for p in range(P):
                nc.gpsimd.dma_start(sg_sbs[p][:, :, 0:512], sg[p, :, :, 0:512])
            for p in range(P):
                nc.scalar.dma_start(vkn_sbs[p][:, 0:4, :], vkn[p, :, 0:4, :])
            nc.scalar.dma_start(m01_sb[:], m01d[:])
            for p in range(P):
                nc.sync.dma_start(qkt_sbs[p][:, :, 512:1024],
                                  qkt[p, :, :, 512:1024])
            for p in range(P):
                nc.gpsimd.dma_start(sg_sbs[p][:, :, 512:1280],
                                    sg[p, :, :, 512:1280])
            for p in range(P):
                nc.scalar.dma_start(vkn_sbs[p][:, 4:10, :], vkn[p, :, 4:10, :])
            for p in range(P):
                nc.sync.dma_start(qkt_sbs[p][:, :, 1024:1536],
                                  qkt[p, :, :, 1024:1536])
            for p in range(P):
                nc.gpsimd.dma_start(sg_sbs[p][:, :, 1280:2048],
                                    sg[p, :, :, 1280:2048])
            for p in range(P):
                nc.scalar.dma_start(vkn_sbs[p][:, 10:16, :],
                                    vkn[p, :, 10:16, :])
            for p in range(P):
                nc.sync.dma_start(qkt_sbs[p][:, :, 1536:2048],
                                  qkt[p, :, :, 1536:2048])

            def kt(p):
                return qkt_sbs[p][:, 0, :]

            def qt(p):
                return qkt_sbs[p][:, 1, :]

            def sgk(p):
                return sg_sbs[p][:, 0, :]

            def sgq(p):
                return sg_sbs[p][:, 1, :]

            def vv_ap(p, i):
                return vkn_sbs[p][:, i, 0:65]

            def sgkn_ap(p, c):
                return vkn_sbs[p][:, c, 65:129]

            s_cur = [None, None]
            prev = [None, None]
            for n in range(NB):
                qw = 256 if n < NB - 1 else 128
                c0, c1 = n * 128, (n + 1) * 128
                units = TAIL_SCHED[n]

                # -- 1. band score MMs, both pairs into one bank --
                bandt = bandp.tile([128, 512], F32, tag="bandt")
                for p in range(P):
                    nc.tensor.matmul(bandt[:, 256 * p:256 * p + qw],
                                     kt(p)[:, c0:c1], qt(p)[:, c0:c0 + qw],
                                     start=True, stop=True,
                                     skip_group_check=True)
                # -- 2. sg + kn MMs, both pairs into one bank --
                sknt = sknp.tile([128, 512], F32, tag="sknt")
                for p in range(P):
                    nc.tensor.matmul(sknt[:, 128 * p:128 * (p + 1)],
                                     sgk(p)[:, c0:c1], sgq(p)[:, c0:c1],
                                     start=True, stop=True,
                                     skip_group_check=True)
                for p in range(P):
                    nc.tensor.matmul(sknt[0:64, 256 + 65 * p:321 + 65 * p],
                                     sgkn_ap(p, n), vv_ap(p, n),
                                     start=True, stop=True,
                                     skip_group_check=True)
                # -- 3. first tail unit MMs --
                tptiles = []
                for tb, unit in units:
                    lo = 0 if unit == 1 else 1024
                    hi = min(tb * 128, 1024) if unit == 1 else tb * 128
                    for p in range(P):
                        tp_ = tailp.tile([128, 1024], F32, tag="tp")
                        for off in range(lo, hi, 512):
                            n_ = min(512, hi - off)
                            nc.tensor.matmul(tp_[:, off - lo:off - lo + n_],
                                             qt(p)[:, tb * 128:tb * 128 + 128],
                                             kt(p)[:, off:off + n_],
                                             start=True, stop=True)
                        tptiles.append((tb, unit, p, tp_, hi - lo))
                    if unit == 1 and len(units) > 1:
                        break   # issue remaining units after the out MMs
                # -- 4. trailing output MMs for block n-1 --
                if prev[0] is not None:
                    pn = n - 1
                    # p1 first: its stat was produced later, so the first MM's
                    # wait covers both (minimal sem count on the PE queue)
                    for p in (1, 0):
                        pot = prev[p]["ot"]
                        nc.tensor.matmul(pot[:], vv_ap(p, pn),
                                         prev[p]["stat"][:], start=True,
                                         stop=(pn == 0), skip_group_check=True)
                    if pn > 0:
                        for p in (1, 0):
                            pot = prev[p]["ot"]
                            nc.tensor.matmul(pot[:, 256:384],
                                             prev[p]["s_before"][:],
                                             sgq(p)[:, pn * 128:pn * 128 + 128],
                                             start=False, stop=True,
                                             skip_group_check=True)
                # -- 5. remaining tail unit MMs --
                done_units = {(tb, u) for tb, u, _, _, _ in tptiles}
                for tb, unit in units:
                    if (tb, unit) in done_units:
                        continue
                    lo = 0 if unit == 1 else 1024
                    hi = min(tb * 128, 1024) if unit == 1 else tb * 128
                    for p in range(P):
                        tp_ = tailp.tile([128, 1024], F32, tag="tp")
                        for off in range(lo, hi, 512):
                            n_ = min(512, hi - off)
                            nc.tensor.matmul(tp_[:, off - lo:off - lo + n_],
                                             qt(p)[:, tb * 128:tb * 128 + 128],
                                             kt(p)[:, off:off + n_],
                                             start=True, stop=True)
                        tptiles.append((tb, unit, p, tp_, hi - lo))
                # -- 6. ACT: ONE merged band exp, then tail exps --
                st_e = workp.tile([128, 512], BF16, tag="st_e")
                nc.scalar.activation(st_e[:, 0:256 + qw], bandt[:, 0:256 + qw],
                                     AF.Exp, scale=TEMP)
                for tb, unit, p, tp_, w_ in tptiles:
                    acccol = tb if unit == 1 else NB + tb
                    scr = workp.tile([128, 1024], BF16, tag="scr")
                    nc.scalar.activation(scr[:, 0:w_], tp_[:, 0:w_],
                                         AF.Exp, scale=TEMP,
                                         accum_out=tacc[p][:, acccol:acccol + 1])
                # -- 7. DVE: masks + state update, then drain block n-1 --
                stats = []
                for p in range(P):
                    stat = workp.tile([128, 384], BF16, tag="stat")
                    nc.vector.tensor_tensor(stat[:, 0:qw],
                                            st_e[:, 256 * p:256 * p + qw],
                                            m01_sb[:, 0:qw], OP.mult)
                    if qw < 256:
                        nc.vector.memset(stat[:, 128:256], 0.0)
                    nc.vector.tensor_tensor(stat[:, 256:384],
                                            sknt[:, 128 * p:128 * (p + 1)],
                                            m01_sb[:, 0:128], OP.mult)
                    stats.append(stat)
                s_before = [s_cur[0], s_cur[1]]
                for p in range(P):
                    s_nxt = sp.tile([64, 65], BF16, tag=f"s{p}")
                    kn_ap = sknt[0:64, 256 + 65 * p:321 + 65 * p]
                    if n == 0:
                        nc.vector.tensor_copy(s_nxt[:], kn_ap)
                    else:
                        nc.vector.scalar_tensor_tensor(s_nxt[:], s_cur[p][:],
                                                       1.0, kn_ap,
                                                       OP.mult, OP.add)
                    s_cur[p] = s_nxt
                if prev[0] is not None:
                    for p in range(P):
                        nc.vector.tensor_copy(accs[p][:, n - 1, :],
                                              prev[p]["ot"][:])
                # -- 8. bookkeeping + periodic output drain --
                for p in range(P):
                    ot = otp.tile([65, 384], F32, tag="ot")
                    prev[p] = {"ot": ot, "stat": stats[p],
                               "s_before": s_before[p]}
                if n in (5, 9, 13):
                    for p in range(P):
                        nc.gpsimd.dma_start(svlv[p, :, n - 5:n - 1, :],
                                            accs[p][:, n - 5:n - 1, :])
                if n == NB - 1:
                    # all tail accums are complete (last unit ran at iter 14)
                    for p in range(P):
                        nc.sync.dma_start(tails[p], tacc[p][:])
                        nc.gpsimd.dma_start(svlv[p, :, 12:15, :],
                                            accs[p][:, 12:15, :])

            # ---- epilogue: finish last block for both pairs ----
            pn = NB - 1
            for p in range(P):
                pot = prev[p]["ot"]
                nc.tensor.matmul(pot[:], vv_ap(p, pn), prev[p]["stat"][:],
                                 start=True, stop=False, skip_group_check=True)
            for p in range(P):
                pot = prev[p]["ot"]
                nc.tensor.matmul(pot[:, 256:384], prev[p]["s_before"][:],
                                 sgq(p)[:, pn * 128:pn * 128 + 128],
                                 start=False, stop=True, skip_group_check=True)
            for p in range(P):
                nc.vector.tensor_copy(accs[p][:, pn, :], prev[p]["ot"][:])
            # final drain split across two DGEs so the transfers overlap
            nc.scalar.dma_start(svlv[0, :, 15:NB, :], accs[0][:, 15:NB, :])
            nc.sync.dma_start(svlv[1, :, 15:NB, :], accs[1][:, 15:NB, :])

    nc.compile()
    return nc


def host_prep(queries, keys, values, key_lengths_mask, blend):
    """Build per-core in_maps from full inputs."""
    q = np.ascontiguousarray(np.transpose(queries, (0, 2, 1, 3)))  # [N,H,L,E]
    k = np.ascontiguousarray(np.transpose(keys, (0, 2, 1, 3)))
    v = np.ascontiguousarray(np.transpose(values, (0, 2, 1, 3)))
    q = q.reshape(N * H, L, E).astype(np.float32)
    k = k.reshape(N * H, L, E).astype(np.float32)
    v = v.reshape(N * H, L, E).astype(np.float32)
    klm = np.asarray(key_lengths_mask, np.float32)  # [N, L]

    ii = np.arange(128)[:, None]
    cc = np.arange(256)[None, :]
    m01 = ((cc - ii >= 0) & (cc - ii <= 128)).astype(np.float32)

    in_maps = []
    for core in range(NCORES):
        qkts, sgs, vkns = [], [], []
        for p in range(PAIRS_PER_CORE):
            g = core * PAIRS_PER_CORE + p
            n = g // H
            qg, kg, vg = q[g], k[g], v[g]          # [L, E]
            kl = klm[n]                             # [L]
            i01 = (kl > 0).astype(np.float32)

            qkt_p = np.empty((65, 2, L), np.float32)
            qkt_p[0:64, 0] = kg.T
            qkt_p[64, 0] = -1e9 * (1.0 - i01)
            qkt_p[0:64, 1] = qg.T
            qkt_p[64, 1] = 1.0

            phiq = np.tanh(qg) + 1.0
            phik = np.tanh(kg) + 1.0
            sg_p = np.empty((64, 2, L), np.float32)
            sg_p[:, 0] = phik.T
            sg_p[:, 1] = phiq.T

            vv_full = np.empty((L, 65), np.float32)
            vv_full[:, 0:64] = vg * kl[:, None]
            vv_full[:, 64] = kl
            vkn_p = np.empty((128, NB, 129), np.float32)
            vkn_p[:, :, 0:65] = vv_full.reshape(NB, 128, 65).transpose(1, 0, 2)
            vkn_p[:, :, 65:129] = phik.reshape(NB, 128, 64).transpose(1, 0, 2)

            qkts.append(qkt_p.astype(ml_dtypes.bfloat16))
            sgs.append(sg_p.astype(ml_dtypes.bfloat16))
            vkns.append(vkn_p.astype(ml_dtypes.bfloat16))

        in_maps.append({
            "qkt": np.ascontiguousarray(np.stack(qkts)),
            "sg": np.ascontiguousarray(np.stack(sgs)),
            "vkn": np.ascontiguousarray(np.stack(vkns)),
            "m01d": np.ascontiguousarray(m01.astype(ml_dtypes.bfloat16)),
        })
    return in_maps


def assemble(results, blend):
    """Normalize + blend on host from raw numerators/denominators."""
    b = float(np.asarray(blend).reshape(-1)[0])
    full = np.empty((N, H, L, E), np.float32)
    for core in range(NCORES):
        r = results[core]
        svlv = np.asarray(r["svlv"], dtype=np.float32)   # [P, 65, NB, 384]
        tails = np.asarray(r["tails"])                   # [P, 128, 2*NB]
        for p in range(PAIRS_PER_CORE):
            g = core * PAIRS_PER_CORE + p
            n, h = g // H, g % H
            sv = svlv[p, :, :, 0:256]       # [65, block, 256]
            lv = svlv[p, :, :, 256:384]     # [65, chunk, 128]
            tl_sum = tails[p, :, 0:NB] + tails[p, :, NB:2 * NB]
            den = tl_sum.T + sv[64, :, 0:128]            # [NB, 128]
            num = sv[0:64, :, 0:128].copy()              # [64, NB, 128]
            num[:, 1:, :] += sv[0:64, 0:NB - 1, 128:256]
            lvn = lv[0:64]                               # [64, NB, 128]
            lvd = lv[64]                                 # [NB, 128]
            o = (b * num / den[None] +
                 (1.0 - b) * lvn / (lvd[None] + EPS))    # [64, NB, 128]
            full[n, h] = o.transpose(1, 2, 0).reshape(L, E)
    return np.ascontiguousarray(np.transpose(full, (0, 2, 1, 3)))


def kernel(queries, keys, values, key_lengths_mask, blend, _trace=False):
    if "nc" not in _cached:
        _cached["nc"] = build_nc()
    nc = _cached["nc"]
    in_maps = host_prep(queries, keys, values, key_lengths_mask, blend)
    res = bass_utils.run_bass_kernel_spmd(nc, in_maps, core_ids=list(range(NCORES)),
                                          trace=_trace)
    _cached["last_results"] = res
    return assemble(res.results, blend)
